# revision 1
# baseline (speedup 1.0000x reference)
"""Binarized 3x3 conv + batchnorm(train) + sign, on 8 TRN2 NeuronCores.

Math: out = sign((y - mean)/sqrt(var+eps)) where y = conv(x, sign(w)) + sign(b)
and mean/var are per-channel batch stats.  Since sqrt(var+eps) > 0, the output
is exactly sign(y - mean_c): variance never needs to be computed.

Strategy (data-parallel over batch, 4 images/core):
 - implicit GEMM: for each of 9 (kh,kw) shifts and 2 input-channel blocks,
   accumulate W[128ci,128co].T @ x_shifted[128ci, 504px] into PSUM.
   Rows are packed 9-at-a-time (9*56=504 free dim); the 2 wrap-around columns
   per row are discarded at PSUM->SBUF drain time.
 - fp32-quality precision from a split matmul: x = hi + lo with hi = fp16(x)
   (full-rate on the PE) and lo = (x - hi)*4096 in fp8-e4m3 run in DoubleRow
   perf mode (half-cycle per row, both ci blocks folded into one matmul).
   Weights are exactly +-1 in fp16/fp8.  The drain combines
   y = hi_psum + lo_psum/4096 + bias and harvests per-channel sums for free
   (ScalarE activation / DVE tensor_tensor_reduce accumulators).
 - one tiny AllReduce (128x2 fp32) across the 8 cores for the global mean.
 - pass 2: t=(y+(-mean))>=0 on VectorE, out=2t-1 on ScalarE, DMA out as bf16
   (+-1 exact), host converts to fp32.
"""

import sys

if "/opt/trn_rl_repo" not in sys.path:
    sys.path.insert(0, "/opt/trn_rl_repo")

import numpy as np
import ml_dtypes

N_CORES = 8
N_PER_CORE = 4          # images per core
CI = 256                # in channels
CO = 256                # out channels
H = W = 56
OH = OW = 54
HWF = H * W             # 3136
HWPAD_H = HWF + 4       # fp16 hi image length in SBUF (pad for row overhang)
HWPAD_L = HWF + 16      # fp8 lo image length; pair-dim stride must be 16B-aligned
NPIX = OH * OW          # 2916
RT = 6                  # row tiles per image (9 rows each)
RROWS = 9
FREE = RROWS * W        # 504 matmul free dim
TFREE = RROWS * OW      # 486 valid outputs per tile
N_TOT = N_CORES * N_PER_CORE
MEAN_SCALE = 1.0 / (N_TOT * NPIX)
LO_SCALE = 4096.0       # lo stored as (x - fp16(x)) * LO_SCALE in fp8 e4m3
RTG = 3                 # rt tiles per psum group (3 hi + 3 lo banks)

BF16 = ml_dtypes.bfloat16
FP8 = ml_dtypes.float8_e4m3


def build(nc, n_cores=N_CORES):
    """Emit the SPMD program into a bacc.Bacc instance."""
    import concourse.mybir as mybir
    from concourse import tile

    f32 = mybir.dt.float32
    f16 = mybir.dt.float16
    bf16 = mybir.dt.bfloat16
    fp8 = mybir.dt.float8e4
    ACT = mybir.ActivationFunctionType
    DR = mybir.MatmulPerfMode.DoubleRow

    xh_d = nc.dram_tensor("xh", [N_PER_CORE, 128, 2, HWPAD_H], f16, kind="ExternalInput")
    xl_d = nc.dram_tensor("xl", [N_PER_CORE, 128, 2, HWPAD_L], fp8, kind="ExternalInput")
    w_d = nc.dram_tensor("wt", [128, 2 * 2 * 9 * 128], f16, kind="ExternalInput")
    w8_d = nc.dram_tensor("w8", [128, 2 * 9 * 2 * 128], fp8, kind="ExternalInput")
    y_d = nc.dram_tensor("y", [N_PER_CORE, 2, 128, NPIX], mybir.dt.uint8, kind="ExternalOutput")

    n_tiles = N_PER_CORE * 2 * RT  # 48

    with tile.TileContext(nc) as tc:
        with (
            tc.tile_pool(name="wpool", bufs=1) as wpool,
            tc.tile_pool(name="xpool", bufs=2) as xpool,
            tc.tile_pool(name="ypool", bufs=1) as ypool,
            tc.tile_pool(name="spool", bufs=1) as spool,
            tc.tile_pool(name="opool", bufs=4) as opool,
            tc.tile_pool(name="tpool", bufs=6) as tpool,
            tc.tile_pool(name="pspool", bufs=8, space="PSUM") as pspool,
            tc.tile_pool(name="drampool", bufs=2, space="DRAM") as drampool,
        ):
            w_sb = wpool.tile([128, 2 * 2 * 9 * 128], f16)
            nc.sync.dma_start(w_sb[:], w_d[:])
            w8_sb = wpool.tile([128, 2, 9, 2, 128], fp8)
            nc.sync.dma_start(
                w8_sb[:],
                w8_d[:].rearrange("p (c s b m) -> p c s b m", c=2, s=9, b=2),
            )
            y_sb = ypool.tile([128, n_tiles * TFREE], f32)
            sums = spool.tile([128, n_tiles], f32, tag="sums")

            # ---------------- phase 1: conv + drain (+bias, +sums) ----------
            for n in range(N_PER_CORE):
                xh_sb = xpool.tile([128, 2, HWPAD_H], f16, tag="xh")
                xl_sb = xpool.tile([128, 2, HWPAD_L], fp8, tag="xl")
                nc.sync.dma_start(xh_sb[:], xh_d[n])
                nc.sync.dma_start(xl_sb[:], xl_d[n])

                for cb in range(2):
                    for rtg in range(RT // RTG):
                        rts = [rtg * RTG + i for i in range(RTG)]
                        hps = [
                            pspool.tile([128, TFREE], f32, tag="ps", name=f"hps{i}")
                            for i in range(RTG)
                        ]
                        lps = [
                            pspool.tile([128, FREE], f32, tag="ps", name=f"lps{i}")
                            for i in range(RTG)
                        ]
                        # lo pass first: fp8 DoubleRow, both ci blocks per
                        # matmul, s-outer so each DR weight load feeds RTG MMs
                        for s in range(9):
                            kh, kw = divmod(s, 3)
                            lw8 = w8_sb[:, cb, s]
                            for i, rt in enumerate(rts):
                                off = (rt * RROWS + kh) * W + kw
                                nc.tensor.matmul(
                                    lps[i][:],
                                    lw8,
                                    xl_sb[:, :, off : off + FREE],
                                    start=(s == 0),
                                    stop=(s == 8),
                                    perf_mode=DR,
                                )
                        # lo drains can run on ScalarE while the hi pass
                        # computes, freeing the lo banks early
                        tmps = []
                        for i, rt in enumerate(rts):
                            tmp = tpool.tile([128, TFREE], f32, tag="tmp", name=f"tmp{i}")
                            lps_v = lps[i][:].rearrange("p (r c) -> p r c", c=W)[
                                :, :, 0:OW
                            ]
                            nc.scalar.activation(
                                tmp[:].rearrange("p (r c) -> p r c", c=OW),
                                lps_v,
                                ACT.Copy,
                                scale=1.0 / LO_SCALE,
                            )
                            tmps.append(tmp)
                        # hi pass: fp16, rt-outer (FWL makes the extra weight
                        # loads ~free) so tile i's drain overlaps tile i+1's MMs
                        for i, rt in enumerate(rts):
                            for b in range(2):
                                for s in range(9):
                                    kh, kw = divmod(s, 3)
                                    k = ((b * 2 + cb) * 9 + s) * 128
                                    lw = w_sb[:, k : k + 128]
                                    first = b == 0 and s == 0
                                    last = b == 1 and s == 8
                                    off = (rt * RROWS + kh) * W + kw
                                    rhs = xh_sb[:, b, off : off + FREE].rearrange(
                                        "p (r c) -> p r c", c=W
                                    )[:, :, 0:OW]
                                    nc.tensor.matmul(
                                        hps[i][:],
                                        lw,
                                        rhs,
                                        start=first,
                                        stop=last,
                                    )
                            # drain: y = hi + lo/LO_SCALE ; accum channel sums
                            # (the +-1 channel bias cancels exactly in
                            # sign(y-mean), so it is dropped)
                            t = (cb * N_PER_CORE + n) * RT + rt
                            nc.vector.scalar_tensor_tensor(
                                y_sb[:, t * TFREE : (t + 1) * TFREE],
                                hps[i][:],
                                1.0,
                                tmps[i][:],
                                mybir.AluOpType.mult,
                                mybir.AluOpType.add,
                                accum_out=sums[:, t : t + 1],
                            )

            # ---------------- phase 2: global mean via AllReduce ------------
            sums2 = spool.tile([128, 2], f32, tag="sums2")
            # tile index t = (cb*N + n)*RT + rt, so cb is outermost:
            # one X-axis reduce over the 24 per-cb columns.
            nc.vector.tensor_reduce(
                sums2[:],
                sums[:].rearrange("p (c m) -> p c m", c=2),
                axis=mybir.AxisListType.X,
                op=mybir.AluOpType.add,
            )
            sums_g = spool.tile([128, 2], f32, tag="sumsg")
            if n_cores > 1:
                cc_in = drampool.tile([128, 2], f32)
                cc_out = drampool.tile([128, 2], f32)
                nc.sync.dma_start(cc_in[:], sums2[:])
                nc.gpsimd.collective_compute(
                    "AllReduce",
                    mybir.AluOpType.add,
                    replica_groups=[list(range(n_cores))],
                    ins=[cc_in.opt()],
                    outs=[cc_out.opt()],
                )
                nc.sync.dma_start(sums_g[:], cc_out[:])
            else:
                # single-core timing variant (TimelineSim can't model
                # collectives): mean is just this core's sums
                nc.vector.tensor_copy(sums_g[:], sums2[:])
            neg_mean = spool.tile([128, 2], f32, tag="negmean")
            nc.scalar.mul(neg_mean[:], sums_g[:], -MEAN_SCALE)

            # ---------------- phase 3: binarize + store ---------------------
            # bin = (y + (-mean)) >= 0 in {0,1} bf16; host maps to +-1 fp32.
            # One DVE op + one DMA per (cb, n) image-block (6 rt tiles = the
            # whole contiguous [128, 2916] slab).
            # bin = (y + (-mean)) >= 0 in {0,1} uint8 (1B/elem halves the
            # output DMA); host maps to +-1 fp32.
            for cb in range(2):
                for n in range(N_PER_CORE):
                    t0 = (cb * N_PER_CORE + n) * RT
                    bin_t = opool.tile([128, RT * TFREE], mybir.dt.uint8, tag="bin")
                    nc.vector.tensor_scalar(
                        bin_t[:],
                        y_sb[:, t0 * TFREE : (t0 + RT) * TFREE],
                        neg_mean[:, cb : cb + 1],
                        0.0,
                        mybir.AluOpType.add,
                        mybir.AluOpType.is_ge,
                    )
                    nc.sync.dma_start(y_d[n, cb], bin_t[:])

    nc.compile()
    return nc


def prep_inputs(x, weight, bias):
    """Host-side shard + layout prep. Returns list of 8 per-core input maps."""
    assert x.shape == (N_TOT, CI, H, W) and x.dtype == np.float32

    # x -> [core, n, p, b, hw]; hi = fp16(x), lo = (x - hi)*LO_SCALE in fp8
    xs = x.reshape(N_CORES, N_PER_CORE, 2, 128, HWF).transpose(0, 1, 3, 2, 4)
    xh = xs.astype(np.float16)
    xl = ((xs - xh.astype(np.float32)) * np.float32(LO_SCALE)).astype(FP8)
    xh = np.pad(xh, ((0, 0),) * 4 + ((0, HWPAD_H - HWF),))
    xl = np.pad(xl, ((0, 0),) * 4 + ((0, HWPAD_L - HWF),))

    wb = np.where(weight >= 0, np.float32(1.0), np.float32(-1.0))
    # [cb, co_f, b, ci_p, kh, kw] -> hi: [ci_p, b, cb, (kh kw), co_f]
    w6 = wb.reshape(2, 128, 2, 128, 3, 3)
    wt = (
        w6.transpose(3, 2, 0, 4, 5, 1)
        .reshape(128, 2 * 2 * 9 * 128)
        .astype(np.float16)
    )
    # lo: [ci_p, cb, (kh kw), b, co_f]
    w8 = (
        w6.transpose(3, 0, 4, 5, 2, 1)
        .reshape(128, 2 * 9 * 2 * 128)
        .astype(FP8)
    )
    return [
        {
            "xh": np.ascontiguousarray(xh[c]),
            "xl": np.ascontiguousarray(xl[c]),
            "wt": wt,
            "w8": w8,
        }
        for c in range(N_CORES)
    ]


def gather(results):
    """[{y: [4,2,128,2916] uint8 in {0,1}}] * 8 -> (32, 256, 54, 54) fp32 +-1."""
    ys = np.stack([r["y"] for r in results])
    out = ys.astype(np.float32).reshape(N_TOT, CO, OH, OW)
    return out * np.float32(2.0) - np.float32(1.0)


_STATE = {}


def _get_nc():
    if "nc" not in _STATE:
        import concourse.bacc as bacc

        nc = bacc.Bacc(
            "TRN2", target_bir_lowering=False, debug=False, num_devices=N_CORES
        )
        _STATE["nc"] = build(nc)
    return _STATE["nc"]


def kernel(x, weight, bias, _trace=False):
    from concourse.bass_utils import run_bass_kernel_spmd

    nc = _get_nc()
    in_maps = prep_inputs(
        np.asarray(x, np.float32),
        np.asarray(weight, np.float32),
        np.asarray(bias, np.float32),
    )
    res = run_bass_kernel_spmd(
        nc, in_maps, core_ids=list(range(N_CORES)), trace=_trace
    )
    _STATE["last_result"] = res
    return gather(res.results)



# revision 4
# speedup vs baseline: 1.5155x; 1.5155x over previous
"""Binarized 3x3 conv + batchnorm(train) + sign, on 8 TRN2 NeuronCores.

Math: out = sign((y - mean)/sqrt(var+eps)) where y = conv(x, sign(w)) + sign(b)
and mean/var are per-channel batch stats.  Since sqrt(var+eps) > 0, the output
is exactly sign(y - mean_c): variance never needs to be computed.  The +-1
channel bias cancels in sign(y - mean), so it is dropped entirely.

Strategy (data-parallel over batch, 4 images/core):
 - implicit GEMM, all matmuls in fp8-e4m3 DoubleRow perf mode (0.5 PE
   cycles/output-row, 2x the fp16 rate; both 128-ci blocks folded into one
   matmul via the DR pair dim).
 - fp32-quality precision from a 3-component split:
       x ~= c1 + c2/64 + c3/64',  c1 = e4m3(x), c2 = e4m3(64*(x-c1)),
       c3 = e4m3(64*(x - c1 - c2/64))
   The /64 scales are folded into the conv weights: comp-1 weights are +-1,
   comp-2/3 weights are +-2^-6 (exact in e4m3, and +-1-weight products are
   exact sign flips).  All 27 matmuls per output tile (3 comps x 9 taps)
   accumulate into ONE fp32 PSUM bank -> a single drain per tile.
   Measured on the reference inputs: 138/23.9M sign flips (rel err 4.8e-3).
 - per-tile drain copies PSUM->SBUF and harvests per-channel sums for free
   (accum_out), alternating ScalarE / VectorE so neither engine is critical.
 - one tiny AllReduce (128x2 fp32) across the 8 cores for the global mean.
 - pass 2: bin = (y + (-mean)) >= 0 in {0,1} uint8 on VectorE, DMA out;
   host maps to +-1 fp32.
"""

import sys

if "/opt/trn_rl_repo" not in sys.path:
    sys.path.insert(0, "/opt/trn_rl_repo")

import numpy as np
import ml_dtypes

N_CORES = 8
N_PER_CORE = 4          # images per core
CI = 256                # in channels
CO = 256                # out channels
H = W = 56
OH = OW = 54
HWF = H * W             # 3136
HWPAD = HWF + 16        # fp8 image length in SBUF; pair-dim stride 16B-aligned
NPIX = OH * OW          # 2916
RT = 6                  # row tiles per image (9 rows each)
RROWS = 9
FREE = RROWS * W        # 504 raw row span
TFREE = RROWS * OW      # 486 valid outputs per tile
N_TOT = N_CORES * N_PER_CORE
MEAN_SCALE = 1.0 / (N_TOT * NPIX)
C_SCALE = 64.0          # residual components stored at 64x, weights at 1/64
NT = N_PER_CORE * 2 * RT  # 48 tiles per core

FP8 = ml_dtypes.float8_e4m3


def build(nc, n_cores=N_CORES):
    """Emit the SPMD program into a bacc.Bacc instance."""
    import concourse.mybir as mybir
    from concourse import tile

    f32 = mybir.dt.float32
    fp8 = mybir.dt.float8e4
    ACT = mybir.ActivationFunctionType
    DR = mybir.MatmulPerfMode.DoubleRow

    x_d = [
        nc.dram_tensor(f"x{c}", [N_PER_CORE, 128, 2, HWPAD], fp8, kind="ExternalInput")
        for c in range(3)
    ]
    w1_d = nc.dram_tensor("w1", [128, 2, 9, 2, 128], fp8, kind="ExternalInput")
    ws_d = nc.dram_tensor("ws", [128, 2, 9, 2, 128], fp8, kind="ExternalInput")
    y_d = nc.dram_tensor("y", [N_PER_CORE, 2, 128, NPIX], mybir.dt.uint8, kind="ExternalOutput")

    with tile.TileContext(nc) as tc:
        with (
            tc.tile_pool(name="wpool", bufs=1) as wpool,
            tc.tile_pool(name="xpool", bufs=2) as xpool,
            tc.tile_pool(name="ypool", bufs=1) as ypool,
            tc.tile_pool(name="spool", bufs=1) as spool,
            tc.tile_pool(name="opool", bufs=4) as opool,
            tc.tile_pool(name="pspool", bufs=8, space="PSUM") as pspool,
            tc.tile_pool(name="drampool", bufs=2, space="DRAM") as drampool,
        ):
            w1_sb = wpool.tile([128, 2, 9, 2, 128], fp8, tag="w1")
            ws_sb = wpool.tile([128, 2, 9, 2, 128], fp8, tag="ws")
            nc.sync.dma_start(w1_sb[:], w1_d[:])
            nc.sync.dma_start(ws_sb[:], ws_d[:])
            y_sb = ypool.tile([128, NT * TFREE], f32)
            sums = spool.tile([128, NT], f32, tag="sums")

            # ---------------- phase 1: conv + drain (+sums) ------------------
            for n in range(N_PER_CORE):
                xc = [
                    xpool.tile([128, 2, HWPAD], fp8, tag=f"x{c}", name=f"x{c}")
                    for c in range(3)
                ]
                for c in range(3):
                    nc.sync.dma_start(xc[c][:], x_d[c][n])

                for cb in range(2):
                    for rt in range(RT):
                        ps = pspool.tile([128, FREE], f32, tag="ps")
                        for c in range(3):
                            w_sb = w1_sb if c == 0 else ws_sb
                            for s in range(9):
                                kh, kw = divmod(s, 3)
                                off = (rt * RROWS + kh) * W + kw
                                nc.tensor.matmul(
                                    ps[:],
                                    w_sb[:, cb, s],
                                    xc[c][:, :, off : off + FREE],
                                    start=(c == 0 and s == 0),
                                    stop=(c == 2 and s == 8),
                                    perf_mode=DR,
                                )
                        t = (cb * N_PER_CORE + n) * RT + rt
                        # drain the 54 valid of each 56-col row (wrap cols
                        # discarded), harvesting the per-channel sum
                        ps_v = ps[:].rearrange("p (r c) -> p r c", c=W)[:, :, 0:OW]
                        osl = y_sb[:, t * TFREE : (t + 1) * TFREE].rearrange(
                            "p (r c) -> p r c", c=OW
                        )
                        if t % 2 == 0:
                            nc.scalar.activation(
                                osl, ps_v, ACT.Copy, accum_out=sums[:, t : t + 1]
                            )
                        else:
                            nc.vector.tensor_scalar(
                                osl,
                                ps_v,
                                0.0,
                                0.0,
                                mybir.AluOpType.add,
                                mybir.AluOpType.add,
                                accum_out=sums[:, t : t + 1],
                            )

            # ---------------- phase 2: global mean via AllReduce ------------
            sums2 = spool.tile([128, 2], f32, tag="sums2")
            # tile index t = (cb*N + n)*RT + rt, so cb is outermost:
            # one X-axis reduce over the 24 per-cb columns.
            nc.vector.tensor_reduce(
                sums2[:],
                sums[:].rearrange("p (c m) -> p c m", c=2),
                axis=mybir.AxisListType.X,
                op=mybir.AluOpType.add,
            )
            sums_g = spool.tile([128, 2], f32, tag="sumsg")
            if n_cores > 1:
                cc_in = drampool.tile([128, 2], f32)
                cc_out = drampool.tile([128, 2], f32)
                nc.sync.dma_start(cc_in[:], sums2[:])
                nc.gpsimd.collective_compute(
                    "AllReduce",
                    mybir.AluOpType.add,
                    replica_groups=[list(range(n_cores))],
                    ins=[cc_in.opt()],
                    outs=[cc_out.opt()],
                )
                nc.sync.dma_start(sums_g[:], cc_out[:])
            else:
                # single-core timing variant (TimelineSim can't model
                # collectives): mean is just this core's sums
                nc.vector.tensor_copy(sums_g[:], sums2[:])
            neg_mean = spool.tile([128, 2], f32, tag="negmean")
            nc.scalar.mul(neg_mean[:], sums_g[:], -MEAN_SCALE)

            # ---------------- phase 3: binarize + store ---------------------
            # bin = (y + (-mean)) >= 0 in {0,1} uint8 (1B/elem output DMA);
            # host maps to +-1 fp32.
            for cb in range(2):
                for n in range(N_PER_CORE):
                    t0 = (cb * N_PER_CORE + n) * RT
                    bin_t = opool.tile([128, RT * TFREE], mybir.dt.uint8, tag="bin")
                    nc.vector.tensor_scalar(
                        bin_t[:],
                        y_sb[:, t0 * TFREE : (t0 + RT) * TFREE],
                        neg_mean[:, cb : cb + 1],
                        0.0,
                        mybir.AluOpType.add,
                        mybir.AluOpType.is_ge,
                    )
                    nc.sync.dma_start(y_d[n, cb], bin_t[:])

    nc.compile()
    return nc


def prep_inputs(x, weight, bias):
    """Host-side shard + layout prep. Returns list of 8 per-core input maps."""
    assert x.shape == (N_TOT, CI, H, W) and x.dtype == np.float32

    # x -> [core, n, ci_f(p), ci_b, hw]; 3-component e4m3 split
    xs = np.ascontiguousarray(
        x.reshape(N_CORES, N_PER_CORE, 2, 128, HWF).transpose(0, 1, 3, 2, 4)
    )
    c1 = xs.astype(FP8)
    r1 = xs - c1.astype(np.float32)
    c2 = (r1 * np.float32(C_SCALE)).astype(FP8)
    r2 = r1 - c2.astype(np.float32) * np.float32(1.0 / C_SCALE)
    c3 = (r2 * np.float32(C_SCALE)).astype(FP8)
    pad = ((0, 0),) * 4 + ((0, HWPAD - HWF),)
    c1 = np.pad(c1, pad)
    c2 = np.pad(c2, pad)
    c3 = np.pad(c3, pad)

    wb = np.where(weight >= 0, np.float32(1.0), np.float32(-1.0))
    # [co_b, co_f, ci_b, ci_f, kh, kw] -> [ci_f(p), co_b, (kh kw), ci_b, co_f]
    w6 = wb.reshape(2, 128, 2, 128, 3, 3)
    wt = np.ascontiguousarray(w6.transpose(3, 0, 4, 5, 2, 1)).reshape(
        128, 2, 9, 2, 128
    )
    w1 = wt.astype(FP8)
    ws = (wt * np.float32(1.0 / C_SCALE)).astype(FP8)  # +-2^-6, exact
    return [
        {
            "x0": c1[c],
            "x1": c2[c],
            "x2": c3[c],
            "w1": w1,
            "ws": ws,
        }
        for c in range(N_CORES)
    ]


def gather(results):
    """[{y: [4,2,128,2916] uint8 in {0,1}}] * 8 -> (32, 256, 54, 54) fp32 +-1."""
    ys = np.stack([r["y"] for r in results])
    out = ys.astype(np.float32).reshape(N_TOT, CO, OH, OW)
    return out * np.float32(2.0) - np.float32(1.0)


_STATE = {}


def _get_nc():
    if "nc" not in _STATE:
        import concourse.bacc as bacc

        nc = bacc.Bacc(
            "TRN2", target_bir_lowering=False, debug=False, num_devices=N_CORES
        )
        _STATE["nc"] = build(nc)
    return _STATE["nc"]


def kernel(x, weight, bias, _trace=False):
    from concourse.bass_utils import run_bass_kernel_spmd

    nc = _get_nc()
    in_maps = prep_inputs(
        np.asarray(x, np.float32),
        np.asarray(weight, np.float32),
        np.asarray(bias, np.float32),
    )
    res = run_bass_kernel_spmd(
        nc, in_maps, core_ids=list(range(N_CORES)), trace=_trace
    )
    _STATE["last_result"] = res
    return gather(res.results)


# revision 22
# speedup vs baseline: 1.5467x; 1.0206x over previous
"""Binarized 3x3 conv + batchnorm(train) + sign, on 8 TRN2 NeuronCores.

Math: out = sign((y - mean)/sqrt(var+eps)) where y = conv(x, sign(w)) + sign(b)
and mean/var are per-channel batch stats.  Since sqrt(var+eps) > 0, the output
is exactly sign(y - mean_c): variance never needs to be computed.  The +-1
channel bias cancels in sign(y - mean), so it is dropped entirely.

Strategy (data-parallel over batch, 4 images/core):
 - implicit GEMM, all matmuls in fp8-e4m3 DoubleRow perf mode (0.5 PE
   cycles/output-row, 2x the fp16 rate; both 128-ci blocks folded into one
   matmul via the DR pair dim).
 - fp32-quality precision from a 3-component split:
       x ~= c1 + c2/64 + c3/64',  c1 = e4m3(x), c2 = e4m3(64*(x-c1)),
       c3 = e4m3(64*(x - c1 - c2/64))
   The /64 scales are folded into the conv weights: comp-1 weights are +-1,
   comp-2/3 weights are +-2^-6 (exact in e4m3, and +-1-weight products are
   exact sign flips).  All 27 matmuls per output tile (3 comps x 9 taps)
   accumulate into ONE fp32 PSUM bank -> a single drain per tile.
   Measured on the reference inputs: 138/23.9M sign flips (rel err 4.8e-3).
 - per-tile drain copies PSUM->SBUF and harvests per-channel sums for free
   (accum_out), alternating ScalarE / VectorE so neither engine is critical.
 - one tiny AllReduce (128x2 fp32) across the 8 cores for the global mean.
 - pass 2: bin = (y + (-mean)) >= 0 in {0,1} uint8 on VectorE, DMA out;
   host maps to +-1 fp32.
"""

import sys

if "/opt/trn_rl_repo" not in sys.path:
    sys.path.insert(0, "/opt/trn_rl_repo")

import numpy as np
import ml_dtypes

N_CORES = 8
N_PER_CORE = 4          # images per core
CI = 256                # in channels
CO = 256                # out channels
H = W = 56
OH = OW = 54
HWF = H * W             # 3136
HWPAD = HWF + 16        # fp8 image length in SBUF; pair-dim stride 16B-aligned
NPIX = OH * OW          # 2916
RT = 6                  # row tiles per image (9 rows each)
RROWS = 9
FREE = RROWS * W        # 504 raw row span
TFREE = RROWS * OW      # 486 valid outputs per tile
N_TOT = N_CORES * N_PER_CORE
MEAN_SCALE = 1.0 / (N_TOT * NPIX)
C_SCALE = 64.0          # residual components stored at 64x, weights at 1/64
NT = N_PER_CORE * 2 * RT  # 48 tiles per core

# engine per phase-3 binarize block (cb*4+n): v=DVE is_ge {0,1},
# a=ScalarE Sign {-1,0,1}; all stored as fp8 bytes
BIN_ENG = ["v", "a", "v", "a", "v", "a", "v", "v"]

FP8 = ml_dtypes.float8_e4m3


def build(nc, n_cores=N_CORES):
    """Emit the SPMD program into a bacc.Bacc instance."""
    import concourse.mybir as mybir
    from concourse import tile

    f32 = mybir.dt.float32
    fp8 = mybir.dt.float8e4
    ACT = mybir.ActivationFunctionType
    DR = mybir.MatmulPerfMode.DoubleRow

    x_d = [
        nc.dram_tensor(f"x{c}", [N_PER_CORE, 128, 2, HWPAD], fp8, kind="ExternalInput")
        for c in range(3)
    ]
    w1_d = nc.dram_tensor("w1", [128, 2, 9, 2, 128], fp8, kind="ExternalInput")
    ws_d = nc.dram_tensor("ws", [128, 2, 9, 2, 128], fp8, kind="ExternalInput")
    y_d = nc.dram_tensor("y", [N_PER_CORE, 2, 128, NPIX], mybir.dt.uint8, kind="ExternalOutput")

    with tile.TileContext(nc) as tc:
        with (
            tc.tile_pool(name="wpool", bufs=1) as wpool,
            tc.tile_pool(name="xpool", bufs=2) as xpool,
            tc.tile_pool(name="ypool", bufs=1) as ypool,
            tc.tile_pool(name="spool", bufs=1) as spool,
            tc.tile_pool(name="opool", bufs=4) as opool,
            tc.tile_pool(name="pspool", bufs=8, space="PSUM") as pspool,
            tc.tile_pool(name="drampool", bufs=2, space="DRAM") as drampool,
        ):
            w1_sb = wpool.tile([128, 2, 9, 2, 128], fp8, tag="w1")
            ws_sb = wpool.tile([128, 2, 9, 2, 128], fp8, tag="ws")
            y_sb = ypool.tile([128, NT * TFREE], f32)
            sums = spool.tile([128, NT], f32, tag="sums")

            # ---------------- phase 1: conv + drain (+sums) ------------------
            # All DMA transfers serialize on the HWDGE device, so the startup
            # transfers are ordered by first use: w1[cb0] + head of x0 (first
            # 27 matmuls), then ws[cb0]+x1 head, x2 head, tails, cb1 weights.
            # The component loop is OUTER within each 3-tile psum group so the
            # first 27 matmuls depend only on w1+x0.
            HD = 30 * W  # 1680: covers rt 0-2 matmul reads (rows 0..29)
            for n in range(N_PER_CORE):
                xc = [
                    xpool.tile([128, 2, HWPAD], fp8, tag=f"x{c}", name=f"x{c}")
                    for c in range(3)
                ]
                if n == 0:
                    nc.sync.dma_start(w1_sb[:, 0], w1_d[:, 0])
                    nc.scalar.dma_start(xc[0][:, :, 0:HD], x_d[0][n][:, :, 0:HD])
                    nc.sync.dma_start(ws_sb[:, 0], ws_d[:, 0])
                    nc.scalar.dma_start(xc[1][:, :, 0:HD], x_d[1][n][:, :, 0:HD])
                    nc.sync.dma_start(xc[0][:, :, HD:], x_d[0][n][:, :, HD:])
                    nc.scalar.dma_start(xc[2][:, :, 0:HD], x_d[2][n][:, :, 0:HD])
                    nc.sync.dma_start(xc[1][:, :, HD:], x_d[1][n][:, :, HD:])
                    nc.scalar.dma_start(xc[2][:, :, HD:], x_d[2][n][:, :, HD:])
                    nc.sync.dma_start(w1_sb[:, 1], w1_d[:, 1])
                    nc.scalar.dma_start(ws_sb[:, 1], ws_d[:, 1])
                else:
                    nc.scalar.dma_start(xc[0][:], x_d[0][n])
                    nc.gpsimd.dma_start(xc[1][:], x_d[1][n])
                    nc.gpsimd.dma_start(xc[2][:], x_d[2][n])

                for cb in range(2):
                    for rt in range(RT):
                        ps = pspool.tile([128, FREE], f32, tag="ps")
                        for c in range(3):
                            w_sb = w1_sb if c == 0 else ws_sb
                            for s in range(9):
                                kh, kw = divmod(s, 3)
                                off = (rt * RROWS + kh) * W + kw
                                nc.tensor.matmul(
                                    ps[:],
                                    w_sb[:, cb, s],
                                    xc[c][:, :, off : off + FREE],
                                    start=(c == 0 and s == 0),
                                    stop=(c == 2 and s == 8),
                                    perf_mode=DR,
                                )
                        t = (cb * N_PER_CORE + n) * RT + rt
                        # drain the 54 valid of each 56-col row (wrap cols
                        # discarded), harvesting the per-channel sum
                        ps_v = ps[:].rearrange("p (r c) -> p r c", c=W)[:, :, 0:OW]
                        osl = y_sb[:, t * TFREE : (t + 1) * TFREE].rearrange(
                            "p (r c) -> p r c", c=OW
                        )
                        if t % 2 == 0:
                            nc.scalar.activation(
                                osl, ps_v, ACT.Copy, accum_out=sums[:, t : t + 1]
                            )
                        else:
                            nc.vector.tensor_scalar(
                                osl,
                                ps_v,
                                0.0,
                                0.0,
                                mybir.AluOpType.add,
                                mybir.AluOpType.add,
                                accum_out=sums[:, t : t + 1],
                            )

            # ---------------- phase 2: global mean via AllReduce ------------
            sums2 = spool.tile([128, 2], f32, tag="sums2")
            # tile index t = (cb*N + n)*RT + rt, so cb is outermost:
            # one X-axis reduce over the 24 per-cb columns.
            nc.vector.tensor_reduce(
                sums2[:],
                sums[:].rearrange("p (c m) -> p c m", c=2),
                axis=mybir.AxisListType.X,
                op=mybir.AluOpType.add,
            )
            neg_mean = spool.tile([128, 2], f32, tag="negmean")
            if n_cores > 1:
                cc_in = drampool.tile([128, 2], f32)
                cc_out = drampool.tile([128, 2], f32)
                nc.sync.dma_start(cc_in[:], sums2[:])
                nc.gpsimd.collective_compute(
                    "AllReduce",
                    mybir.AluOpType.add,
                    replica_groups=[list(range(n_cores))],
                    ins=[cc_in.opt()],
                    outs=[cc_out.opt()],
                )
                sums_g = spool.tile([128, 2], f32, tag="sumsg")
                nc.sync.dma_start(sums_g[:], cc_out[:])
                nc.scalar.mul(neg_mean[:], sums_g[:], -MEAN_SCALE)
            else:
                # single-core timing variant (TimelineSim can't model
                # collectives): mean is just this core's sums
                nc.scalar.mul(neg_mean[:], sums2[:], -MEAN_SCALE)

            # ---------------- phase 3: binarize + store ---------------------
            # spread the 8 blocks over DVE (is_ge -> {0,1}), ScalarE
            # (Sign -> {-1,0,1}) and GpSimd (is_ge) per BIN_ENG; all 1B fp8
            # out.  Host maps back to +-1 fp32 per block encoding.
            # out-DMAs issue from SP/GpSimd so their issue cost never
            # interleaves with the ScalarE Sign ops
            for cb in range(2):
                for n in range(N_PER_CORE):
                    b = cb * N_PER_CORE + n
                    t0 = b * RT
                    eng = BIN_ENG[b]
                    bin_t = opool.tile([128, RT * TFREE], fp8, tag="bin")
                    y_v = y_sb[:, t0 * TFREE : (t0 + RT) * TFREE]
                    nm = neg_mean[:, cb : cb + 1]
                    if eng == "a":
                        nc.scalar.activation(bin_t[:], y_v, ACT.Sign, bias=nm)
                    else:
                        nc.vector.tensor_scalar(
                            bin_t[:],
                            y_v,
                            nm,
                            0.0,
                            mybir.AluOpType.add,
                            mybir.AluOpType.is_ge,
                        )
                    dma_e = nc.sync if b < 4 else nc.gpsimd
                    dma_e.dma_start(y_d[n, cb], bin_t[:].bitcast(mybir.dt.uint8))

    nc.compile()
    return nc


def prep_inputs(x, weight, bias):
    """Host-side shard + layout prep. Returns list of 8 per-core input maps."""
    assert x.shape == (N_TOT, CI, H, W) and x.dtype == np.float32

    # x -> [core, n, ci_f(p), ci_b, hw]; 3-component e4m3 split
    xs = np.ascontiguousarray(
        x.reshape(N_CORES, N_PER_CORE, 2, 128, HWF).transpose(0, 1, 3, 2, 4)
    )
    c1 = xs.astype(FP8)
    r1 = xs - c1.astype(np.float32)
    c2 = (r1 * np.float32(C_SCALE)).astype(FP8)
    r2 = r1 - c2.astype(np.float32) * np.float32(1.0 / C_SCALE)
    c3 = (r2 * np.float32(C_SCALE)).astype(FP8)
    pad = ((0, 0),) * 4 + ((0, HWPAD - HWF),)
    c1 = np.pad(c1, pad)
    c2 = np.pad(c2, pad)
    c3 = np.pad(c3, pad)

    wb = np.where(weight >= 0, np.float32(1.0), np.float32(-1.0))
    # [co_b, co_f, ci_b, ci_f, kh, kw] -> [ci_f(p), co_b, (kh kw), ci_b, co_f]
    w6 = wb.reshape(2, 128, 2, 128, 3, 3)
    wt = np.ascontiguousarray(w6.transpose(3, 0, 4, 5, 2, 1)).reshape(
        128, 2, 9, 2, 128
    )
    w1 = wt.astype(FP8)
    ws = (wt * np.float32(1.0 / C_SCALE)).astype(FP8)  # +-2^-6, exact
    return [
        {
            "x0": c1[c],
            "x1": c2[c],
            "x2": c3[c],
            "w1": w1,
            "ws": ws,
        }
        for c in range(N_CORES)
    ]


def gather(results):
    """[{y: [4,2,128,2916] fp8}] * 8 -> (32, 256, 54, 54) fp32 +-1.

    DVE/GpSimd blocks hold {0,1} (is_ge), ScalarE blocks hold {-1,0,1}
    (Sign); see BIN_ENG."""
    ys = np.stack([np.asarray(r["y"]).view(FP8) for r in results]).astype(np.float32)
    out = np.empty_like(ys)
    for b, eng in enumerate(BIN_ENG):
        cb, n = divmod(b, N_PER_CORE)
        v = ys[:, n, cb]
        if eng == "a":
            out[:, n, cb] = np.where(v > 0, np.float32(1.0), np.float32(-1.0))
        else:
            out[:, n, cb] = v * np.float32(2.0) - np.float32(1.0)
    return out.reshape(N_TOT, CO, OH, OW)


_STATE = {}


def _get_nc():
    if "nc" not in _STATE:
        import concourse.bacc as bacc

        nc = bacc.Bacc(
            "TRN2", target_bir_lowering=False, debug=False, num_devices=N_CORES
        )
        _STATE["nc"] = build(nc)
    return _STATE["nc"]


def kernel(x, weight, bias, _trace=False):
    from concourse.bass_utils import run_bass_kernel_spmd

    nc = _get_nc()
    in_maps = prep_inputs(
        np.asarray(x, np.float32),
        np.asarray(weight, np.float32),
        np.asarray(bias, np.float32),
    )
    res = run_bass_kernel_spmd(
        nc, in_maps, core_ids=list(range(N_CORES)), trace=_trace
    )
    _STATE["last_result"] = res
    return gather(res.results)


# revision 23
# speedup vs baseline: 1.5492x; 1.0016x over previous
"""Binarized 3x3 conv + batchnorm(train) + sign, on 8 TRN2 NeuronCores.

Math: out = sign((y - mean)/sqrt(var+eps)) where y = conv(x, sign(w)) + sign(b)
and mean/var are per-channel batch stats.  Since sqrt(var+eps) > 0, the output
is exactly sign(y - mean_c): variance never needs to be computed.  The +-1
channel bias cancels in sign(y - mean), so it is dropped entirely.

Strategy (data-parallel over batch, 4 images/core):
 - implicit GEMM, all matmuls in fp8-e4m3 DoubleRow perf mode (0.5 PE
   cycles/output-row, 2x the fp16 rate; both 128-ci blocks folded into one
   matmul via the DR pair dim).
 - fp32-quality precision from a 3-component split:
       x ~= c1 + c2/64 + c3/64',  c1 = e4m3(x), c2 = e4m3(64*(x-c1)),
       c3 = e4m3(64*(x - c1 - c2/64))
   The /64 scales are folded into the conv weights: comp-1 weights are +-1,
   comp-2/3 weights are +-2^-6 (exact in e4m3, and +-1-weight products are
   exact sign flips).  All 27 matmuls per output tile (3 comps x 9 taps)
   accumulate into ONE fp32 PSUM bank -> a single drain per tile.
   Measured on the reference inputs: 138/23.9M sign flips (rel err 4.8e-3).
 - per-tile drain copies PSUM->SBUF and harvests per-channel sums for free
   (accum_out), alternating ScalarE / VectorE so neither engine is critical.
 - one tiny AllReduce (128x2 fp32) across the 8 cores for the global mean.
 - pass 2: bin = (y + (-mean)) >= 0 in {0,1} uint8 on VectorE, DMA out;
   host maps to +-1 fp32.
"""

import sys

if "/opt/trn_rl_repo" not in sys.path:
    sys.path.insert(0, "/opt/trn_rl_repo")

import numpy as np
import ml_dtypes

N_CORES = 8
N_PER_CORE = 4          # images per core
CI = 256                # in channels
CO = 256                # out channels
H = W = 56
OH = OW = 54
HWF = H * W             # 3136
HWPAD = HWF + 16        # fp8 image length in SBUF; pair-dim stride 16B-aligned
NPIX = OH * OW          # 2916
RT = 6                  # row tiles per image (9 rows each)
RROWS = 9
FREE = RROWS * W        # 504 raw row span
TFREE = RROWS * OW      # 486 valid outputs per tile
N_TOT = N_CORES * N_PER_CORE
MEAN_SCALE = 1.0 / (N_TOT * NPIX)
C_SCALE = 64.0          # residual components stored at 64x, weights at 1/64
NT = N_PER_CORE * 2 * RT  # 48 tiles per core

# engine per phase-3 binarize block (cb*4+n): v=DVE is_ge {0,1},
# a=ScalarE Sign {-1,0,1}; all stored as fp8 bytes
BIN_ENG = ["v", "a", "v", "a", "v", "a", "v", "v"]

FP8 = ml_dtypes.float8_e4m3


def build(nc, n_cores=N_CORES):
    """Emit the SPMD program into a bacc.Bacc instance."""
    import concourse.mybir as mybir
    from concourse import tile

    f32 = mybir.dt.float32
    fp8 = mybir.dt.float8e4
    ACT = mybir.ActivationFunctionType
    DR = mybir.MatmulPerfMode.DoubleRow

    x_d = [
        nc.dram_tensor(f"x{c}", [N_PER_CORE, 128, 2, HWPAD], fp8, kind="ExternalInput")
        for c in range(3)
    ]
    w1_d = nc.dram_tensor("w1", [128, 2, 9, 2, 128], fp8, kind="ExternalInput")
    ws_d = nc.dram_tensor("ws", [128, 2, 9, 2, 128], fp8, kind="ExternalInput")
    y_d = nc.dram_tensor("y", [N_PER_CORE, 2, 128, NPIX], mybir.dt.uint8, kind="ExternalOutput")

    with tile.TileContext(nc) as tc:
        with (
            tc.tile_pool(name="wpool", bufs=1) as wpool,
            tc.tile_pool(name="xpool", bufs=2) as xpool,
            tc.tile_pool(name="ypool", bufs=1) as ypool,
            tc.tile_pool(name="spool", bufs=1) as spool,
            tc.tile_pool(name="opool", bufs=4) as opool,
            tc.tile_pool(name="pspool", bufs=8, space="PSUM") as pspool,
            tc.tile_pool(name="drampool", bufs=2, space="DRAM") as drampool,
        ):
            w1_sb = wpool.tile([128, 2, 9, 2, 128], fp8, tag="w1")
            ws_sb = wpool.tile([128, 2, 9, 2, 128], fp8, tag="ws")
            y_sb = ypool.tile([128, NT * TFREE], f32)
            sums = spool.tile([128, NT], f32, tag="sums")

            # ---------------- phase 1: conv + drain (+sums) ------------------
            # All DMA transfers serialize on the HWDGE device, so the startup
            # transfers are ordered by first use: w1[cb0] + head of x0 (first
            # 27 matmuls), then ws[cb0]+x1 head, x2 head, tails, cb1 weights.
            # The component loop is OUTER within each 3-tile psum group so the
            # first 27 matmuls depend only on w1+x0.
            HD = 30 * W  # 1680: covers rt 0-2 matmul reads (rows 0..29)
            for n in range(N_PER_CORE):
                xc = [
                    xpool.tile([128, 2, HWPAD], fp8, tag=f"x{c}", name=f"x{c}")
                    for c in range(3)
                ]
                if n == 0:
                    nc.sync.dma_start(w1_sb[:, 0], w1_d[:, 0])
                    nc.scalar.dma_start(xc[0][:, :, 0:HD], x_d[0][n][:, :, 0:HD])
                    nc.sync.dma_start(ws_sb[:, 0], ws_d[:, 0])
                    nc.scalar.dma_start(xc[1][:, :, 0:HD], x_d[1][n][:, :, 0:HD])
                    nc.sync.dma_start(xc[0][:, :, HD:], x_d[0][n][:, :, HD:])
                    nc.scalar.dma_start(xc[2][:, :, 0:HD], x_d[2][n][:, :, 0:HD])
                    nc.sync.dma_start(xc[1][:, :, HD:], x_d[1][n][:, :, HD:])
                    nc.scalar.dma_start(xc[2][:, :, HD:], x_d[2][n][:, :, HD:])
                    nc.sync.dma_start(w1_sb[:, 1], w1_d[:, 1])
                    nc.scalar.dma_start(ws_sb[:, 1], ws_d[:, 1])
                else:
                    nc.scalar.dma_start(xc[0][:], x_d[0][n])
                    nc.gpsimd.dma_start(xc[1][:], x_d[1][n])
                    nc.gpsimd.dma_start(xc[2][:], x_d[2][n])

                for cb in range(2):
                    for rt in range(RT):
                        ps = pspool.tile([128, TFREE], f32, tag="ps")
                        for c in range(3):
                            w_sb = w1_sb if c == 0 else ws_sb
                            for s in range(9):
                                kh, kw = divmod(s, 3)
                                off = (rt * RROWS + kh) * W + kw
                                # 4D rhs view drops the 2 wrap cols per row:
                                # 486-wide DR output (0.5 cyc/row on 486
                                # instead of 504)
                                rhs = xc[c][:, :, off : off + FREE].rearrange(
                                    "p b (r c) -> p b r c", c=W
                                )[:, :, :, 0:OW]
                                nc.tensor.matmul(
                                    ps[:],
                                    w_sb[:, cb, s],
                                    rhs,
                                    start=(c == 0 and s == 0),
                                    stop=(c == 2 and s == 8),
                                    perf_mode=DR,
                                )
                        t = (cb * N_PER_CORE + n) * RT + rt
                        ps_v = ps[:]
                        osl = y_sb[:, t * TFREE : (t + 1) * TFREE]
                        if t % 2 == 0:
                            nc.scalar.activation(
                                osl, ps_v, ACT.Copy, accum_out=sums[:, t : t + 1]
                            )
                        else:
                            nc.vector.tensor_scalar(
                                osl,
                                ps_v,
                                0.0,
                                0.0,
                                mybir.AluOpType.add,
                                mybir.AluOpType.add,
                                accum_out=sums[:, t : t + 1],
                            )

            # ---------------- phase 2: global mean via AllReduce ------------
            sums2 = spool.tile([128, 2], f32, tag="sums2")
            # tile index t = (cb*N + n)*RT + rt, so cb is outermost:
            # one X-axis reduce over the 24 per-cb columns.
            nc.vector.tensor_reduce(
                sums2[:],
                sums[:].rearrange("p (c m) -> p c m", c=2),
                axis=mybir.AxisListType.X,
                op=mybir.AluOpType.add,
            )
            neg_mean = spool.tile([128, 2], f32, tag="negmean")
            if n_cores > 1:
                cc_in = drampool.tile([128, 2], f32)
                cc_out = drampool.tile([128, 2], f32)
                nc.sync.dma_start(cc_in[:], sums2[:])
                nc.gpsimd.collective_compute(
                    "AllReduce",
                    mybir.AluOpType.add,
                    replica_groups=[list(range(n_cores))],
                    ins=[cc_in.opt()],
                    outs=[cc_out.opt()],
                )
                sums_g = spool.tile([128, 2], f32, tag="sumsg")
                nc.sync.dma_start(sums_g[:], cc_out[:])
                nc.scalar.mul(neg_mean[:], sums_g[:], -MEAN_SCALE)
            else:
                # single-core timing variant (TimelineSim can't model
                # collectives): mean is just this core's sums
                nc.scalar.mul(neg_mean[:], sums2[:], -MEAN_SCALE)

            # ---------------- phase 3: binarize + store ---------------------
            # spread the 8 blocks over DVE (is_ge -> {0,1}), ScalarE
            # (Sign -> {-1,0,1}) and GpSimd (is_ge) per BIN_ENG; all 1B fp8
            # out.  Host maps back to +-1 fp32 per block encoding.
            # out-DMAs issue from SP/GpSimd so their issue cost never
            # interleaves with the ScalarE Sign ops
            for cb in range(2):
                for n in range(N_PER_CORE):
                    b = cb * N_PER_CORE + n
                    t0 = b * RT
                    eng = BIN_ENG[b]
                    bin_t = opool.tile([128, RT * TFREE], fp8, tag="bin")
                    y_v = y_sb[:, t0 * TFREE : (t0 + RT) * TFREE]
                    nm = neg_mean[:, cb : cb + 1]
                    if eng == "a":
                        nc.scalar.activation(bin_t[:], y_v, ACT.Sign, bias=nm)
                    else:
                        nc.vector.tensor_scalar(
                            bin_t[:],
                            y_v,
                            nm,
                            0.0,
                            mybir.AluOpType.add,
                            mybir.AluOpType.is_ge,
                        )
                    dma_e = nc.sync if b < 4 else nc.gpsimd
                    dma_e.dma_start(y_d[n, cb], bin_t[:].bitcast(mybir.dt.uint8))

    nc.compile()
    return nc


def prep_inputs(x, weight, bias):
    """Host-side shard + layout prep. Returns list of 8 per-core input maps."""
    assert x.shape == (N_TOT, CI, H, W) and x.dtype == np.float32

    # x -> [core, n, ci_f(p), ci_b, hw]; 3-component e4m3 split
    xs = np.ascontiguousarray(
        x.reshape(N_CORES, N_PER_CORE, 2, 128, HWF).transpose(0, 1, 3, 2, 4)
    )
    c1 = xs.astype(FP8)
    r1 = xs - c1.astype(np.float32)
    c2 = (r1 * np.float32(C_SCALE)).astype(FP8)
    r2 = r1 - c2.astype(np.float32) * np.float32(1.0 / C_SCALE)
    c3 = (r2 * np.float32(C_SCALE)).astype(FP8)
    pad = ((0, 0),) * 4 + ((0, HWPAD - HWF),)
    c1 = np.pad(c1, pad)
    c2 = np.pad(c2, pad)
    c3 = np.pad(c3, pad)

    wb = np.where(weight >= 0, np.float32(1.0), np.float32(-1.0))
    # [co_b, co_f, ci_b, ci_f, kh, kw] -> [ci_f(p), co_b, (kh kw), ci_b, co_f]
    w6 = wb.reshape(2, 128, 2, 128, 3, 3)
    wt = np.ascontiguousarray(w6.transpose(3, 0, 4, 5, 2, 1)).reshape(
        128, 2, 9, 2, 128
    )
    w1 = wt.astype(FP8)
    ws = (wt * np.float32(1.0 / C_SCALE)).astype(FP8)  # +-2^-6, exact
    return [
        {
            "x0": c1[c],
            "x1": c2[c],
            "x2": c3[c],
            "w1": w1,
            "ws": ws,
        }
        for c in range(N_CORES)
    ]


def gather(results):
    """[{y: [4,2,128,2916] fp8}] * 8 -> (32, 256, 54, 54) fp32 +-1.

    DVE/GpSimd blocks hold {0,1} (is_ge), ScalarE blocks hold {-1,0,1}
    (Sign); see BIN_ENG."""
    ys = np.stack([np.asarray(r["y"]).view(FP8) for r in results]).astype(np.float32)
    out = np.empty_like(ys)
    for b, eng in enumerate(BIN_ENG):
        cb, n = divmod(b, N_PER_CORE)
        v = ys[:, n, cb]
        if eng == "a":
            out[:, n, cb] = np.where(v > 0, np.float32(1.0), np.float32(-1.0))
        else:
            out[:, n, cb] = v * np.float32(2.0) - np.float32(1.0)
    return out.reshape(N_TOT, CO, OH, OW)


_STATE = {}


def _get_nc():
    if "nc" not in _STATE:
        import concourse.bacc as bacc

        nc = bacc.Bacc(
            "TRN2", target_bir_lowering=False, debug=False, num_devices=N_CORES
        )
        _STATE["nc"] = build(nc)
    return _STATE["nc"]


def kernel(x, weight, bias, _trace=False):
    from concourse.bass_utils import run_bass_kernel_spmd

    nc = _get_nc()
    in_maps = prep_inputs(
        np.asarray(x, np.float32),
        np.asarray(weight, np.float32),
        np.asarray(bias, np.float32),
    )
    res = run_bass_kernel_spmd(
        nc, in_maps, core_ids=list(range(N_CORES)), trace=_trace
    )
    _STATE["last_result"] = res
    return gather(res.results)


# revision 25
# speedup vs baseline: 1.5499x; 1.0005x over previous
"""Binarized 3x3 conv + batchnorm(train) + sign, on 8 TRN2 NeuronCores.

Math: out = sign((y - mean)/sqrt(var+eps)) where y = conv(x, sign(w)) + sign(b)
and mean/var are per-channel batch stats.  Since sqrt(var+eps) > 0, the output
is exactly sign(y - mean_c): variance never needs to be computed.  The +-1
channel bias cancels in sign(y - mean), so it is dropped entirely.

Strategy (data-parallel over batch, 4 images/core):
 - implicit GEMM, all matmuls in fp8-e4m3 DoubleRow perf mode (0.5 PE
   cycles/output-row, 2x the fp16 rate; both 128-ci blocks folded into one
   matmul via the DR pair dim).
 - fp32-quality precision from a 3-component split:
       x ~= c1 + c2/64 + c3/64',  c1 = e4m3(x), c2 = e4m3(64*(x-c1)),
       c3 = e4m3(64*(x - c1 - c2/64))
   The /64 scales are folded into the conv weights: comp-1 weights are +-1,
   comp-2/3 weights are +-2^-6 (exact in e4m3, and +-1-weight products are
   exact sign flips).  All 27 matmuls per output tile (3 comps x 9 taps)
   accumulate into ONE fp32 PSUM bank -> a single drain per tile.
   Measured on the reference inputs: 138/23.9M sign flips (rel err 4.8e-3).
 - per-tile drain on ScalarE copies PSUM->SBUF and harvests per-channel sums
   for free (accum_out), leaving VectorE clear for the binarize pass.
 - one tiny AllReduce (128x2 fp32) across the 8 cores for the global mean.
 - pass 2: binarize split over VectorE (is_ge -> {0,1}) and ScalarE
   (Sign -> {-1,0,1}) per BIN_ENG, DMA out as 1 byte/elem; host maps each
   block back to +-1 fp32.
"""

import sys

if "/opt/trn_rl_repo" not in sys.path:
    sys.path.insert(0, "/opt/trn_rl_repo")

import numpy as np
import ml_dtypes

N_CORES = 8
N_PER_CORE = 4          # images per core
CI = 256                # in channels
CO = 256                # out channels
H = W = 56
OH = OW = 54
HWF = H * W             # 3136
HWPAD = HWF + 16        # fp8 image length in SBUF; pair-dim stride 16B-aligned
NPIX = OH * OW          # 2916
RT = 6                  # row tiles per image (9 rows each)
RROWS = 9
FREE = RROWS * W        # 504 raw row span
TFREE = RROWS * OW      # 486 valid outputs per tile
N_TOT = N_CORES * N_PER_CORE
MEAN_SCALE = 1.0 / (N_TOT * NPIX)
C_SCALE = 64.0          # residual components stored at 64x, weights at 1/64
NT = N_PER_CORE * 2 * RT  # 48 tiles per core

# engine per phase-3 binarize block (cb*4+n): v=DVE is_ge {0,1},
# a=ScalarE Sign {-1,0,1}; all stored as fp8 bytes
BIN_ENG = ["v", "a", "v", "a", "v", "a", "v", "v"]

FP8 = ml_dtypes.float8_e4m3


def build(nc, n_cores=N_CORES):
    """Emit the SPMD program into a bacc.Bacc instance."""
    import concourse.mybir as mybir
    from concourse import tile

    f32 = mybir.dt.float32
    fp8 = mybir.dt.float8e4
    ACT = mybir.ActivationFunctionType
    DR = mybir.MatmulPerfMode.DoubleRow

    x_d = [
        nc.dram_tensor(f"x{c}", [N_PER_CORE, 128, 2, HWPAD], fp8, kind="ExternalInput")
        for c in range(3)
    ]
    w1_d = nc.dram_tensor("w1", [128, 2, 9, 2, 128], fp8, kind="ExternalInput")
    ws_d = nc.dram_tensor("ws", [128, 2, 9, 2, 128], fp8, kind="ExternalInput")
    y_d = nc.dram_tensor("y", [N_PER_CORE, 2, 128, NPIX], mybir.dt.uint8, kind="ExternalOutput")

    with tile.TileContext(nc) as tc:
        with (
            tc.tile_pool(name="wpool", bufs=1) as wpool,
            tc.tile_pool(name="xpool", bufs=2) as xpool,
            tc.tile_pool(name="ypool", bufs=1) as ypool,
            tc.tile_pool(name="spool", bufs=1) as spool,
            tc.tile_pool(name="opool", bufs=4) as opool,
            tc.tile_pool(name="pspool", bufs=8, space="PSUM") as pspool,
            tc.tile_pool(name="drampool", bufs=2, space="DRAM") as drampool,
        ):
            w1_sb = wpool.tile([128, 2, 9, 2, 128], fp8, tag="w1")
            ws_sb = wpool.tile([128, 2, 9, 2, 128], fp8, tag="ws")
            y_sb = ypool.tile([128, NT * TFREE], f32)
            sums = spool.tile([128, NT], f32, tag="sums")

            # ---------------- phase 1: conv + drain (+sums) ------------------
            # All DMA transfers serialize on the HWDGE device, so the startup
            # transfers are ordered by first use: w1[cb0] + head of x0 (first
            # 27 matmuls), then ws[cb0]+x1 head, x2 head, tails, cb1 weights.
            # The component loop is OUTER within each 3-tile psum group so the
            # first 27 matmuls depend only on w1+x0.
            HD = 30 * W  # 1680: covers rt 0-2 matmul reads (rows 0..29)
            for n in range(N_PER_CORE):
                xc = [
                    xpool.tile([128, 2, HWPAD], fp8, tag=f"x{c}", name=f"x{c}")
                    for c in range(3)
                ]
                if n == 0:
                    nc.sync.dma_start(w1_sb[:, 0], w1_d[:, 0])
                    nc.scalar.dma_start(xc[0][:, :, 0:HD], x_d[0][n][:, :, 0:HD])
                    nc.sync.dma_start(ws_sb[:, 0], ws_d[:, 0])
                    nc.scalar.dma_start(xc[1][:, :, 0:HD], x_d[1][n][:, :, 0:HD])
                    nc.sync.dma_start(xc[0][:, :, HD:], x_d[0][n][:, :, HD:])
                    nc.scalar.dma_start(xc[2][:, :, 0:HD], x_d[2][n][:, :, 0:HD])
                    nc.sync.dma_start(xc[1][:, :, HD:], x_d[1][n][:, :, HD:])
                    nc.scalar.dma_start(xc[2][:, :, HD:], x_d[2][n][:, :, HD:])
                    nc.sync.dma_start(w1_sb[:, 1], w1_d[:, 1])
                    nc.scalar.dma_start(ws_sb[:, 1], ws_d[:, 1])
                else:
                    nc.scalar.dma_start(xc[0][:], x_d[0][n])
                    nc.gpsimd.dma_start(xc[1][:], x_d[1][n])
                    nc.gpsimd.dma_start(xc[2][:], x_d[2][n])

                for cb in range(2):
                    for rt in range(RT):
                        ps = pspool.tile([128, TFREE], f32, tag="ps")
                        for c in range(3):
                            w_sb = w1_sb if c == 0 else ws_sb
                            for s in range(9):
                                kh, kw = divmod(s, 3)
                                off = (rt * RROWS + kh) * W + kw
                                # 4D rhs view drops the 2 wrap cols per row:
                                # 486-wide DR output (0.5 cyc/row on 486
                                # instead of 504)
                                rhs = xc[c][:, :, off : off + FREE].rearrange(
                                    "p b (r c) -> p b r c", c=W
                                )[:, :, :, 0:OW]
                                nc.tensor.matmul(
                                    ps[:],
                                    w_sb[:, cb, s],
                                    rhs,
                                    start=(c == 0 and s == 0),
                                    stop=(c == 2 and s == 8),
                                    perf_mode=DR,
                                )
                        t = (cb * N_PER_CORE + n) * RT + rt
                        ps_v = ps[:]
                        osl = y_sb[:, t * TFREE : (t + 1) * TFREE]
                        nc.scalar.activation(
                            osl, ps_v, ACT.Copy, accum_out=sums[:, t : t + 1]
                        )

            # ---------------- phase 2: global mean via AllReduce ------------
            sums2 = spool.tile([128, 2], f32, tag="sums2")
            # tile index t = (cb*N + n)*RT + rt, so cb is outermost:
            # one X-axis reduce over the 24 per-cb columns.
            nc.vector.tensor_reduce(
                sums2[:],
                sums[:].rearrange("p (c m) -> p c m", c=2),
                axis=mybir.AxisListType.X,
                op=mybir.AluOpType.add,
            )
            neg_mean = spool.tile([128, 2], f32, tag="negmean")
            if n_cores > 1:
                cc_in = drampool.tile([128, 2], f32)
                cc_out = drampool.tile([128, 2], f32)
                nc.sync.dma_start(cc_in[:], sums2[:])
                nc.gpsimd.collective_compute(
                    "AllReduce",
                    mybir.AluOpType.add,
                    replica_groups=[list(range(n_cores))],
                    ins=[cc_in.opt()],
                    outs=[cc_out.opt()],
                )
                sums_g = spool.tile([128, 2], f32, tag="sumsg")
                nc.sync.dma_start(sums_g[:], cc_out[:])
                nc.scalar.mul(neg_mean[:], sums_g[:], -MEAN_SCALE)
            else:
                # single-core timing variant (TimelineSim can't model
                # collectives): mean is just this core's sums
                nc.scalar.mul(neg_mean[:], sums2[:], -MEAN_SCALE)

            # ---------------- phase 3: binarize + store ---------------------
            # spread the 8 blocks over DVE (is_ge -> {0,1}), ScalarE
            # (Sign -> {-1,0,1}) and GpSimd (is_ge) per BIN_ENG; all 1B fp8
            # out.  Host maps back to +-1 fp32 per block encoding.
            # out-DMAs issue from SP/GpSimd so their issue cost never
            # interleaves with the ScalarE Sign ops
            for cb in range(2):
                for n in range(N_PER_CORE):
                    b = cb * N_PER_CORE + n
                    t0 = b * RT
                    eng = BIN_ENG[b]
                    bin_t = opool.tile([128, RT * TFREE], fp8, tag="bin")
                    y_v = y_sb[:, t0 * TFREE : (t0 + RT) * TFREE]
                    nm = neg_mean[:, cb : cb + 1]
                    if eng == "a":
                        nc.scalar.activation(bin_t[:], y_v, ACT.Sign, bias=nm)
                    else:
                        nc.vector.tensor_scalar(
                            bin_t[:],
                            y_v,
                            nm,
                            0.0,
                            mybir.AluOpType.add,
                            mybir.AluOpType.is_ge,
                        )
                    dma_e = nc.sync if b < 4 else nc.gpsimd
                    dma_e.dma_start(y_d[n, cb], bin_t[:].bitcast(mybir.dt.uint8))

    nc.compile()
    return nc


def prep_inputs(x, weight, bias):
    """Host-side shard + layout prep. Returns list of 8 per-core input maps."""
    assert x.shape == (N_TOT, CI, H, W) and x.dtype == np.float32

    # x -> [core, n, ci_f(p), ci_b, hw]; 3-component e4m3 split
    xs = np.ascontiguousarray(
        x.reshape(N_CORES, N_PER_CORE, 2, 128, HWF).transpose(0, 1, 3, 2, 4)
    )
    c1 = xs.astype(FP8)
    r1 = xs - c1.astype(np.float32)
    c2 = (r1 * np.float32(C_SCALE)).astype(FP8)
    r2 = r1 - c2.astype(np.float32) * np.float32(1.0 / C_SCALE)
    c3 = (r2 * np.float32(C_SCALE)).astype(FP8)
    pad = ((0, 0),) * 4 + ((0, HWPAD - HWF),)
    c1 = np.pad(c1, pad)
    c2 = np.pad(c2, pad)
    c3 = np.pad(c3, pad)

    wb = np.where(weight >= 0, np.float32(1.0), np.float32(-1.0))
    # [co_b, co_f, ci_b, ci_f, kh, kw] -> [ci_f(p), co_b, (kh kw), ci_b, co_f]
    w6 = wb.reshape(2, 128, 2, 128, 3, 3)
    wt = np.ascontiguousarray(w6.transpose(3, 0, 4, 5, 2, 1)).reshape(
        128, 2, 9, 2, 128
    )
    w1 = wt.astype(FP8)
    ws = (wt * np.float32(1.0 / C_SCALE)).astype(FP8)  # +-2^-6, exact
    return [
        {
            "x0": c1[c],
            "x1": c2[c],
            "x2": c3[c],
            "w1": w1,
            "ws": ws,
        }
        for c in range(N_CORES)
    ]


def gather(results):
    """[{y: [4,2,128,2916] fp8}] * 8 -> (32, 256, 54, 54) fp32 +-1.

    DVE/GpSimd blocks hold {0,1} (is_ge), ScalarE blocks hold {-1,0,1}
    (Sign); see BIN_ENG."""
    ys = np.stack([np.asarray(r["y"]).view(FP8) for r in results]).astype(np.float32)
    out = np.empty_like(ys)
    for b, eng in enumerate(BIN_ENG):
        cb, n = divmod(b, N_PER_CORE)
        v = ys[:, n, cb]
        if eng == "a":
            out[:, n, cb] = np.where(v > 0, np.float32(1.0), np.float32(-1.0))
        else:
            out[:, n, cb] = v * np.float32(2.0) - np.float32(1.0)
    return out.reshape(N_TOT, CO, OH, OW)


_STATE = {}


def _get_nc():
    if "nc" not in _STATE:
        import concourse.bacc as bacc

        nc = bacc.Bacc(
            "TRN2", target_bir_lowering=False, debug=False, num_devices=N_CORES
        )
        _STATE["nc"] = build(nc)
    return _STATE["nc"]


def kernel(x, weight, bias, _trace=False):
    from concourse.bass_utils import run_bass_kernel_spmd

    nc = _get_nc()
    in_maps = prep_inputs(
        np.asarray(x, np.float32),
        np.asarray(weight, np.float32),
        np.asarray(bias, np.float32),
    )
    res = run_bass_kernel_spmd(
        nc, in_maps, core_ids=list(range(N_CORES)), trace=_trace
    )
    _STATE["last_result"] = res
    return gather(res.results)


# revision 26
# speedup vs baseline: 1.5673x; 1.0112x over previous
"""Binarized 3x3 conv + batchnorm(train) + sign, on 8 TRN2 NeuronCores.

Math: out = sign((y - mean)/sqrt(var+eps)) where y = conv(x, sign(w)) + sign(b)
and mean/var are per-channel batch stats.  Since sqrt(var+eps) > 0, the output
is exactly sign(y - mean_c): variance never needs to be computed.  The +-1
channel bias cancels in sign(y - mean), so it is dropped entirely.

Strategy (data-parallel over batch, 4 images/core):
 - implicit GEMM, all matmuls in fp8-e4m3 DoubleRow perf mode (0.5 PE
   cycles/output-row, 2x the fp16 rate; both 128-ci blocks folded into one
   matmul via the DR pair dim).
 - fp32-quality precision from a 3-component split:
       x ~= c1 + c2/64 + c3/64',  c1 = e4m3(x), c2 = e4m3(64*(x-c1)),
       c3 = e4m3(64*(x - c1 - c2/64))
   The /64 scales are folded into the conv weights: comp-1 weights are +-1,
   comp-2/3 weights are +-2^-6 (exact in e4m3, and +-1-weight products are
   exact sign flips).  All 27 matmuls per output tile (3 comps x 9 taps)
   accumulate into ONE fp32 PSUM bank -> a single drain per tile.
   Measured on the reference inputs: 138/23.9M sign flips (rel err 4.8e-3).
 - per-tile drain on ScalarE copies PSUM->SBUF and harvests per-channel sums
   for free (accum_out), leaving VectorE clear for the binarize pass.
 - one tiny AllReduce (128x2 fp32) across the 8 cores for the global mean.
 - pass 2: binarize split over VectorE (is_ge -> {0,1}) and ScalarE
   (Sign -> {-1,0,1}) per BIN_ENG, DMA out as 1 byte/elem; host maps each
   block back to +-1 fp32.
"""

import sys

if "/opt/trn_rl_repo" not in sys.path:
    sys.path.insert(0, "/opt/trn_rl_repo")

import numpy as np
import ml_dtypes

N_CORES = 8
N_PER_CORE = 4          # images per core
CI = 256                # in channels
CO = 256                # out channels
H = W = 56
OH = OW = 54
HWF = H * W             # 3136
HWPAD = HWF + 16        # fp8 image length in SBUF; pair-dim stride 16B-aligned
NPIX = OH * OW          # 2916
RT = 6                  # row tiles per image (9 rows each)
RROWS = 9
FREE = RROWS * W        # 504 raw row span
TFREE = RROWS * OW      # 486 valid outputs per tile
N_TOT = N_CORES * N_PER_CORE
MEAN_SCALE = 1.0 / (N_TOT * NPIX)
C_SCALE = 64.0          # residual components stored at 64x, weights at 1/64
NT = N_PER_CORE * 2 * RT  # 48 tiles per core

# engine per phase-3 binarize block (cb*4+n): v=DVE is_ge {0,1},
# a=ScalarE Sign {-1,0,1}; all stored as fp8 bytes
BIN_ENG = ["v", "a", "v", "a", "v", "a", "v", "v"]

FP8 = ml_dtypes.float8_e4m3


def build(nc, n_cores=N_CORES):
    """Emit the SPMD program into a bacc.Bacc instance."""
    import concourse.mybir as mybir
    from concourse import tile

    f32 = mybir.dt.float32
    fp8 = mybir.dt.float8e4
    ACT = mybir.ActivationFunctionType
    DR = mybir.MatmulPerfMode.DoubleRow

    x_d = [
        nc.dram_tensor(f"x{c}", [N_PER_CORE, 128, 2, HWPAD], fp8, kind="ExternalInput")
        for c in range(3)
    ]
    w1_d = nc.dram_tensor("w1", [128, 2, 9, 2, 128], fp8, kind="ExternalInput")
    ws_d = nc.dram_tensor("ws", [128, 2, 9, 2, 128], fp8, kind="ExternalInput")
    y_d = nc.dram_tensor("y", [N_PER_CORE, 2, 128, NPIX], mybir.dt.uint8, kind="ExternalOutput")

    with tile.TileContext(nc) as tc:
        with (
            tc.tile_pool(name="wpool", bufs=1) as wpool,
            tc.tile_pool(name="xpool", bufs=2) as xpool,
            tc.tile_pool(name="ypool", bufs=1) as ypool,
            tc.tile_pool(name="spool", bufs=1) as spool,
            tc.tile_pool(name="opool", bufs=8) as opool,
            tc.tile_pool(name="pspool", bufs=8, space="PSUM") as pspool,
            tc.tile_pool(name="drampool", bufs=2, space="DRAM") as drampool,
        ):
            w1_sb = wpool.tile([128, 2, 9, 2, 128], fp8, tag="w1")
            ws_sb = wpool.tile([128, 2, 9, 2, 128], fp8, tag="ws")
            y_sb = ypool.tile([128, NT * TFREE], f32)
            sums = spool.tile([128, NT], f32, tag="sums")

            # ---------------- phase 1: conv + drain (+sums) ------------------
            # All DMA transfers serialize on the HWDGE device, so the startup
            # transfers are ordered by first use: w1[cb0] + head of x0 (first
            # 27 matmuls), then ws[cb0]+x1 head, x2 head, tails, cb1 weights.
            # The component loop is OUTER within each 3-tile psum group so the
            # first 27 matmuls depend only on w1+x0.
            HD = 30 * W  # 1680: covers rt 0-2 matmul reads (rows 0..29)
            for n in range(N_PER_CORE):
                xc = [
                    xpool.tile([128, 2, HWPAD], fp8, tag=f"x{c}", name=f"x{c}")
                    for c in range(3)
                ]
                if n == 0:
                    nc.sync.dma_start(w1_sb[:, 0], w1_d[:, 0])
                    nc.scalar.dma_start(xc[0][:, :, 0:HD], x_d[0][n][:, :, 0:HD])
                    nc.sync.dma_start(ws_sb[:, 0], ws_d[:, 0])
                    nc.scalar.dma_start(xc[1][:, :, 0:HD], x_d[1][n][:, :, 0:HD])
                    nc.sync.dma_start(xc[0][:, :, HD:], x_d[0][n][:, :, HD:])
                    nc.scalar.dma_start(xc[2][:, :, 0:HD], x_d[2][n][:, :, 0:HD])
                    nc.sync.dma_start(xc[1][:, :, HD:], x_d[1][n][:, :, HD:])
                    nc.scalar.dma_start(xc[2][:, :, HD:], x_d[2][n][:, :, HD:])
                    nc.sync.dma_start(w1_sb[:, 1], w1_d[:, 1])
                    nc.scalar.dma_start(ws_sb[:, 1], ws_d[:, 1])
                else:
                    nc.scalar.dma_start(xc[0][:], x_d[0][n])
                    nc.gpsimd.dma_start(xc[1][:], x_d[1][n])
                    nc.gpsimd.dma_start(xc[2][:], x_d[2][n])

                for cb in range(2):
                    for rt in range(RT):
                        ps = pspool.tile([128, TFREE], f32, tag="ps")
                        for c in range(3):
                            w_sb = w1_sb if c == 0 else ws_sb
                            for s in range(9):
                                kh, kw = divmod(s, 3)
                                off = (rt * RROWS + kh) * W + kw
                                # 4D rhs view drops the 2 wrap cols per row:
                                # 486-wide DR output (0.5 cyc/row on 486
                                # instead of 504)
                                rhs = xc[c][:, :, off : off + FREE].rearrange(
                                    "p b (r c) -> p b r c", c=W
                                )[:, :, :, 0:OW]
                                nc.tensor.matmul(
                                    ps[:],
                                    w_sb[:, cb, s],
                                    rhs,
                                    start=(c == 0 and s == 0),
                                    stop=(c == 2 and s == 8),
                                    perf_mode=DR,
                                )
                        t = (cb * N_PER_CORE + n) * RT + rt
                        ps_v = ps[:]
                        osl = y_sb[:, t * TFREE : (t + 1) * TFREE]
                        nc.scalar.activation(
                            osl, ps_v, ACT.Copy, accum_out=sums[:, t : t + 1]
                        )

            # ---------------- phase 2: global mean via AllReduce ------------
            sums2 = spool.tile([128, 2], f32, tag="sums2")
            # tile index t = (cb*N + n)*RT + rt, so cb is outermost:
            # one X-axis reduce over the 24 per-cb columns.
            nc.vector.tensor_reduce(
                sums2[:],
                sums[:].rearrange("p (c m) -> p c m", c=2),
                axis=mybir.AxisListType.X,
                op=mybir.AluOpType.add,
            )
            neg_mean = spool.tile([128, 2], f32, tag="negmean")
            if n_cores > 1:
                cc_in = drampool.tile([128, 2], f32)
                cc_out = drampool.tile([128, 2], f32)
                nc.sync.dma_start(cc_in[:], sums2[:])
                nc.gpsimd.collective_compute(
                    "AllReduce",
                    mybir.AluOpType.add,
                    replica_groups=[list(range(n_cores))],
                    ins=[cc_in.opt()],
                    outs=[cc_out.opt()],
                )
                sums_g = spool.tile([128, 2], f32, tag="sumsg")
                nc.sync.dma_start(sums_g[:], cc_out[:])
                nc.scalar.mul(neg_mean[:], sums_g[:], -MEAN_SCALE)
            else:
                # single-core timing variant (TimelineSim can't model
                # collectives): mean is just this core's sums
                nc.scalar.mul(neg_mean[:], sums2[:], -MEAN_SCALE)

            # ---------------- phase 3: binarize + store ---------------------
            # spread the 8 blocks over DVE (is_ge -> {0,1}), ScalarE
            # (Sign -> {-1,0,1}) and GpSimd (is_ge) per BIN_ENG; all 1B fp8
            # out.  Host maps back to +-1 fp32 per block encoding.
            # out-DMAs issue from SP/GpSimd so their issue cost never
            # interleaves with the ScalarE Sign ops
            for cb in range(2):
                for n in range(N_PER_CORE):
                    b = cb * N_PER_CORE + n
                    t0 = b * RT
                    eng = BIN_ENG[b]
                    bin_t = opool.tile([128, RT * TFREE], fp8, tag="bin")
                    y_v = y_sb[:, t0 * TFREE : (t0 + RT) * TFREE]
                    nm = neg_mean[:, cb : cb + 1]
                    if eng == "a":
                        nc.scalar.activation(bin_t[:], y_v, ACT.Sign, bias=nm)
                    else:
                        nc.vector.tensor_scalar(
                            bin_t[:],
                            y_v,
                            nm,
                            0.0,
                            mybir.AluOpType.add,
                            mybir.AluOpType.is_ge,
                        )
                    dma_e = nc.sync if b < 4 else nc.gpsimd
                    dma_e.dma_start(y_d[n, cb], bin_t[:].bitcast(mybir.dt.uint8))

    nc.compile()
    return nc


def prep_inputs(x, weight, bias):
    """Host-side shard + layout prep. Returns list of 8 per-core input maps."""
    assert x.shape == (N_TOT, CI, H, W) and x.dtype == np.float32

    # x -> [core, n, ci_f(p), ci_b, hw]; 3-component e4m3 split
    xs = np.ascontiguousarray(
        x.reshape(N_CORES, N_PER_CORE, 2, 128, HWF).transpose(0, 1, 3, 2, 4)
    )
    c1 = xs.astype(FP8)
    r1 = xs - c1.astype(np.float32)
    c2 = (r1 * np.float32(C_SCALE)).astype(FP8)
    r2 = r1 - c2.astype(np.float32) * np.float32(1.0 / C_SCALE)
    c3 = (r2 * np.float32(C_SCALE)).astype(FP8)
    pad = ((0, 0),) * 4 + ((0, HWPAD - HWF),)
    c1 = np.pad(c1, pad)
    c2 = np.pad(c2, pad)
    c3 = np.pad(c3, pad)

    wb = np.where(weight >= 0, np.float32(1.0), np.float32(-1.0))
    # [co_b, co_f, ci_b, ci_f, kh, kw] -> [ci_f(p), co_b, (kh kw), ci_b, co_f]
    w6 = wb.reshape(2, 128, 2, 128, 3, 3)
    wt = np.ascontiguousarray(w6.transpose(3, 0, 4, 5, 2, 1)).reshape(
        128, 2, 9, 2, 128
    )
    w1 = wt.astype(FP8)
    ws = (wt * np.float32(1.0 / C_SCALE)).astype(FP8)  # +-2^-6, exact
    return [
        {
            "x0": c1[c],
            "x1": c2[c],
            "x2": c3[c],
            "w1": w1,
            "ws": ws,
        }
        for c in range(N_CORES)
    ]


def gather(results):
    """[{y: [4,2,128,2916] fp8}] * 8 -> (32, 256, 54, 54) fp32 +-1.

    DVE/GpSimd blocks hold {0,1} (is_ge), ScalarE blocks hold {-1,0,1}
    (Sign); see BIN_ENG."""
    ys = np.stack([np.asarray(r["y"]).view(FP8) for r in results]).astype(np.float32)
    out = np.empty_like(ys)
    for b, eng in enumerate(BIN_ENG):
        cb, n = divmod(b, N_PER_CORE)
        v = ys[:, n, cb]
        if eng == "a":
            out[:, n, cb] = np.where(v > 0, np.float32(1.0), np.float32(-1.0))
        else:
            out[:, n, cb] = v * np.float32(2.0) - np.float32(1.0)
    return out.reshape(N_TOT, CO, OH, OW)


_STATE = {}


def _get_nc():
    if "nc" not in _STATE:
        import concourse.bacc as bacc

        nc = bacc.Bacc(
            "TRN2", target_bir_lowering=False, debug=False, num_devices=N_CORES
        )
        _STATE["nc"] = build(nc)
    return _STATE["nc"]


def kernel(x, weight, bias, _trace=False):
    from concourse.bass_utils import run_bass_kernel_spmd

    nc = _get_nc()
    in_maps = prep_inputs(
        np.asarray(x, np.float32),
        np.asarray(weight, np.float32),
        np.asarray(bias, np.float32),
    )
    res = run_bass_kernel_spmd(
        nc, in_maps, core_ids=list(range(N_CORES)), trace=_trace
    )
    _STATE["last_result"] = res
    return gather(res.results)
